# revision 11
# baseline (speedup 1.0000x reference)
"""Trainium2 Bass kernel for multi-head quadratic spatial attention.

Problem: q,k,v [b=8, heads=8, h=32, w=32, d=64] fp32; full attention over
the 1024-position spatial grid independently per (b, head); output
[b, h, w, heads*d].

Sharding: data-parallel over batch — core c handles b=c (8 heads of
[1024, 64] attention per core), no cross-core communication.

Per-core pipeline (heads processed in PAIRS; matmuls bf16 with fp32 PSUM
accumulation):
  - p-major seq tiling (seq = p*8 + t) so every input DMA reads 1-2KB
    contiguous HBM runs per partition; q/k on the sync HWDGE ring, v on
    gpsimd SWDGE; fp32->bf16 cast in-flight
  - ~28 dummy ident matmuls issued up front warm the PE HAM clock gate
    (1.2 -> 2.4 GHz) while the first DMAs land
  - Q,K transposed to d-major via col-tiled PE transposes: head A lands
    on PSUM partitions 0:64 (tile_position (0,0)), head B on 64:128
    ((0,64)) — the two transposes stream concurrently
  - mm1 row-tiled: head A contracts on PE rows 0:64, head B on rows
    64:128 (tile_position (0,0)/(64,0), concurrent) -> St [128,1024] fp32
  - exp split across engines: head A on ScalarE (activation Exp), head B
    on VectorE via the Schraudolph bit-trick (single fused
    tensor_scalar mult+add -> int16, bit-identical to a bf16 exp approx)
  - mm2: lhsT = [V | 1] j-chunk [128, 65] bf16, rhs = Pt slices ->
    accumulate PSUM Ot [65, 512] per i-half; row 64 = softmax sums
  - epilogue in bf16: ot copy on ScalarE, PE transposes back (FWL), one
    batched reciprocal [128,4] + per-block tensor_scalar normalize, bf16
    ostage, DMA upcasts to fp32 on store
"""

from contextlib import ExitStack

import numpy as np

F32 = None
BF16 = None
I16 = None

_cache = {}

# Schraudolph exp in bf16 bit-space: bf16_bits(exp(s*x)) ~= round(x*A + B)
# A = s * 2^7/ln2, B = 2^7*(127 - sigma), sigma = 0.0430 balances the
# piecewise-linear 2^frac error (max rel err ~3%, zero-mean over j which
# mostly cancels in the softmax average).
SCALE = 64.0 ** -0.5
SCHRAUD_A = SCALE * 128.0 / float(np.log(2.0))
SCHRAUD_B = 128.0 * (127.0 - 0.0430)

N_WARM = 28  # dummy matmuls to flip the PE HAM clock gate before real work

# (jb, head-in-pair) St tiles exp'd on VectorE via the Schraudolph bit-trick
# instead of ScalarE's exact exp. Each entry adds ~1/16 of the attention
# weight mass at ~3% weight error (mostly cancelling in the softmax
# average) and takes ~1.1us/pair off the ScalarE critical path.
SCHRAUD_TILES = frozenset()


def _imports():
    global F32, BF16, I16
    import concourse.bass as bass
    import concourse.tile as tile
    from concourse import mybir
    from concourse.masks import make_identity

    F32 = mybir.dt.float32
    BF16 = mybir.dt.bfloat16
    I16 = mybir.dt.int16
    return bass, tile, mybir, make_identity


def _split_multi_waits(nc, mybir):
    """Walrus in this container supports only ONE sync-wait per instruction.
    Hoist extra waits onto same-engine InstNoOp's inserted just before."""
    ctr = 0
    for f in nc.m.functions:
        for bb in f.blocks:
            insts = bb.instructions
            if not any(
                i.sync_info and i.sync_info.on_wait and len(i.sync_info.on_wait) > 1
                for i in insts
            ):
                continue
            out = []
            for inst in insts:
                si = inst.sync_info
                waits = list(si.on_wait) if si and si.on_wait else []
                if len(waits) > 1:
                    for w in waits[:-1]:
                        ctr += 1
                        nop = mybir.InstNoOp(
                            name=f"I-wsplit-{ctr}",
                            engine=inst.engine,
                            ins=[],
                            outs=[],
                            sync_info=mybir.SyncInfo(on_wait=[w], on_update=[]),
                        )
                        nc.register_instruction(nop)
                        out.append(nop)
                    si.on_wait = waits[-1:]
                out.append(inst)
            bb.instructions = out


def _build_nc(heads=8, seq=1024, d=64):
    bass, tile, mybir, make_identity = _imports()
    assert heads % 2 == 0 and seq == 1024 and d == 64
    nt = seq // 128          # 8 blocks of 128 positions
    nh = seq // 512          # 2 i-halves of 512
    dv = d + 1
    TT_MULT = mybir.AluOpType.mult
    TT_ADD = mybir.AluOpType.add

    nc = bass.Bass(trn_type="TRN2", target_bir_lowering=False)
    q_d = nc.dram_tensor("q", [heads, seq, d], F32, kind="ExternalInput")
    k_d = nc.dram_tensor("k", [heads, seq, d], F32, kind="ExternalInput")
    v_d = nc.dram_tensor("v", [heads, seq, d], F32, kind="ExternalInput")
    o_d = nc.dram_tensor("out", [seq, heads * d], F32, kind="ExternalOutput")

    # p-major: seq = p*nt + t -> contiguous 2KB HBM run per partition
    q_ap = q_d[:].rearrange("n (p t) d -> n p t d", p=128)
    k_ap = k_d[:].rearrange("n (p t) d -> n p t d", p=128)
    v_ap = v_d[:].rearrange("n (p t) d -> n p t d", p=128)
    o_ap = o_d[:].rearrange("(p t) c -> p t c", p=128)

    with tile.TileContext(nc) as tc, ExitStack() as ctx:
        consts = ctx.enter_context(tc.tile_pool(name="consts", bufs=1))
        nat = ctx.enter_context(tc.tile_pool(name="nat", bufs=2))
        dmaj = ctx.enter_context(tc.tile_pool(name="dmaj", bufs=2))
        ptp = ctx.enter_context(tc.tile_pool(name="ptp", bufs=36))
        otp = ctx.enter_context(tc.tile_pool(name="otp", bufs=3))
        outp = ctx.enter_context(tc.tile_pool(name="outp", bufs=3))
        small = ctx.enter_context(tc.tile_pool(name="small", bufs=4))

        # PSUM: st 2x4KB (banks 0-3) + oacc 2x2KB (banks 4-5) + tp 4x1KB
        # (banks 6-7) = 16KB
        st_ps = ctx.enter_context(tc.tile_pool(name="st_ps", bufs=2, space="PSUM"))
        oa_ps = ctx.enter_context(tc.tile_pool(name="oa_ps", bufs=2, space="PSUM"))
        tp_ps = ctx.enter_context(tc.tile_pool(name="tp_ps", bufs=2, space="PSUM"))

        ident_bf = consts.tile([128, 128], BF16)
        make_identity(nc, ident_bf[:])

        # Pre-warm the PE HAM clock gate with dependency-free matmuls that
        # run while the first input DMAs are in flight.
        warm = oa_ps.tile([128, 128], F32, tag="oacc")
        for _ in range(N_WARM):
            nc.tensor.matmul(warm[:], ident_bf[:], ident_bf[:], start=True, stop=True)

        def load_and_transpose(pair):
            """DMA pair inputs (bf16 cast, contiguous p-major runs) and build
            packed d-major tiles: head A on partitions 0:64, head B on
            64:128 via col-tiled concurrent PE transposes."""
            st8 = {"heads": (2 * pair, 2 * pair + 1), "v": [], "pts": [],
                   "oacc": {}, "ostage": {}}
            qn, kn = [], []
            for idx, n in enumerate(st8["heads"]):
                qt_n = nat.tile([128, nt, d], BF16, name="q_nat", tag=f"q_nat{idx}")
                kt_n = nat.tile([128, nt, d], BF16, name="k_nat", tag=f"k_nat{idx}")
                qn.append(qt_n)
                kn.append(kt_n)
            # one casting DMA per head-tensor: contiguous 2KB HBM run per
            # partition (gpsimd SWDGE — the only ring that casts)
            for idx, n in enumerate(st8["heads"]):
                nc.gpsimd.dma_start(out=qn[idx][:], in_=q_ap[n])
                nc.gpsimd.dma_start(out=kn[idx][:], in_=k_ap[n])
            for idx, n in enumerate(st8["heads"]):
                vn = nat.tile([128, nt, dv], BF16, name="v_nat", tag=f"v_nat{idx}")
                # ones column for the softmax-denominator trick
                nc.vector.memset(vn[:, :, d : d + 1], 1.0)
                nc.gpsimd.dma_start(out=vn[:, :, 0:d], in_=v_ap[n])
                st8["v"].append(vn)
            qt = dmaj.tile([128, seq], BF16, tag="qt")
            kt = dmaj.tile([128, seq], BF16, tag="kt")
            for g in range(nt // 4):
                for src_pair, dst in ((qn, qt), (kn, kt)):
                    tp = tp_ps.tile([128, 512], BF16, tag="tp")
                    for u in range(4):
                        t = g * 4 + u
                        # col-tiled pair: A -> partitions 0:64 (tile (0,0)),
                        # B -> 64:128 (tile (0,64)); streams overlap
                        nc.tensor.transpose(
                            tp[0:64, u * 128 : (u + 1) * 128],
                            src_pair[0][:, t, :],
                            ident_bf[:],
                        )
                        nc.tensor.transpose(
                            tp[64:128, u * 128 : (u + 1) * 128],
                            src_pair[1][:, t, :],
                            ident_bf[:],
                        )
                    nc.vector.tensor_copy(
                        out=dst[:, g * 512 : (g + 1) * 512], in_=tp[:]
                    )
            st8["qt"], st8["kt"] = qt, kt
            return st8

        def mm1_exp(s, jb):
            """Row-tiled pair mm1 (A on PE rows 0:64, B on rows 64:128,
            concurrent) + exp split across ScalarE (A) / VectorE (B)."""
            qt, kt = s["qt"], s["kt"]
            stA = st_ps.tile([128, seq], F32, name="stA", tag="st")
            stB = st_ps.tile([128, seq], F32, name="stB", tag="st")
            for c in range(nh):
                nc.tensor.matmul(
                    stA[:, c * 512 : (c + 1) * 512],
                    kt[0:64, jb * 128 : (jb + 1) * 128],
                    qt[0:64, c * 512 : (c + 1) * 512],
                    start=True,
                    stop=True,
                )
                nc.tensor.matmul(
                    stB[:, c * 512 : (c + 1) * 512],
                    kt[64:128, jb * 128 : (jb + 1) * 128],
                    qt[64:128, c * 512 : (c + 1) * 512],
                    start=True,
                    stop=True,
                )
            for idx, st in enumerate((stA, stB)):
                pt = ptp.tile([128, seq], BF16, name="pt", tag="pt")
                if (jb, idx) in SCHRAUD_TILES:
                    nc.vector.tensor_scalar(
                        out=pt[:].bitcast(I16),
                        in0=st[:],
                        scalar1=SCHRAUD_A,
                        scalar2=SCHRAUD_B,
                        op0=TT_MULT,
                        op1=TT_ADD,
                    )
                else:
                    nc.scalar.activation(
                        out=pt[:],
                        in_=st[:],
                        func=mybir.ActivationFunctionType.Exp,
                        scale=SCALE,
                    )
                s["pts"].append((jb, idx, pt))

        def mm2_slot(s, slot):
            """One PE-stream slot of the pair's mm2: 4 accumulating
            matmuls of group (head, half) = slot//2; epilogue on the
            closing slot."""
            g = slot // 2
            phase = slot % 2
            idx, half = g // 2, g % 2
            if phase == 0:
                s["oacc"][g] = oa_ps.tile([dv, 512], F32, name="oacc", tag="oacc")
            oacc = s["oacc"][g]
            off = half * 512
            for jj in range(4):
                jb = phase * 4 + jj
                jb2, idx2, pt = s["pts"][jb * 2 + idx]
                assert jb2 == jb and idx2 == idx
                nc.tensor.matmul(
                    oacc[:],
                    s["v"][idx][:, jb, :],
                    pt[:, off : off + 512],
                    start=(jb == 0),
                    stop=(jb == nt - 1),
                )
            if phase == 1:
                _epilogue(s, idx, half, oacc)

        def _epilogue(s, idx, half, oacc):
            n = s["heads"][idx]
            if idx not in s["ostage"]:
                s["ostage"][idx] = outp.tile(
                    [128, nt, d], F32, name="ostage", tag="ostage"
                )
            ostage = s["ostage"][idx]
            ot = otp.tile([dv, 512], BF16, tag="ot")
            nc.vector.tensor_copy(out=ot[:], in_=oacc[:])
            # dv+1 padding keeps each transpose's PSUM write 4-byte aligned
            ob = tp_ps.tile([128, 4, dv + 1], BF16, tag="tp")
            for u in range(4):
                nc.tensor.transpose(
                    ob[:, u, 0:dv], ot[:, u * 128 : (u + 1) * 128], ident_bf[0:dv, 0:dv]
                )
            rec = small.tile([128, 4], F32, tag="rec")
            nc.vector.reciprocal(out=rec[:], in_=ob[:, :, d])
            for u in range(4):
                t = half * 4 + u
                nc.vector.tensor_scalar_mul(
                    ostage[:, t, :], ob[:, u, 0:d], rec[:, u : u + 1]
                )
            if half == nh - 1:
                nc.sync.dma_start(
                    out=o_ap[:, :, n * d : (n + 1) * d], in_=ostage[:]
                )

        # software pipeline: pair p's mm1/exp interleaved with pair p-1's mm2
        prev = None
        for pair in range(heads // 2):
            cur = load_and_transpose(pair)
            for jb in range(nt):
                mm1_exp(cur, jb)
                if prev is not None:
                    mm2_slot(prev, jb)
            prev = cur
        for jb in range(nt):
            mm2_slot(prev, jb)

    _split_multi_waits(nc, mybir)
    return nc


def _get_nc():
    if "nc" not in _cache:
        _cache["nc"] = _build_nc()
    return _cache["nc"]


def _run(q, k, v, trace=False):
    from concourse.bass_utils import run_bass_kernel_spmd

    b, heads, h, w, d = 8, 8, 32, 32, 64
    q = np.ascontiguousarray(np.asarray(q, dtype=np.float32))
    k = np.ascontiguousarray(np.asarray(k, dtype=np.float32))
    v = np.ascontiguousarray(np.asarray(v, dtype=np.float32))
    assert q.shape == (b, heads, h, w, d), q.shape

    nc = _get_nc()
    in_maps = [
        {
            "q": q[c].reshape(heads, h * w, d),
            "k": k[c].reshape(heads, h * w, d),
            "v": v[c].reshape(heads, h * w, d),
        }
        for c in range(b)
    ]
    res = run_bass_kernel_spmd(nc, in_maps, core_ids=list(range(b)), trace=trace)
    out = np.stack(
        [res.results[c]["out"].reshape(h, w, heads * d) for c in range(b)]
    )
    return out, res


def kernel(q, k, v):
    out, _ = _run(q, k, v)
    return out


# revision 19
# speedup vs baseline: 1.1093x; 1.1093x over previous
"""Trainium2 Bass kernel for multi-head quadratic spatial attention.

Problem: q,k,v [b=8, heads=8, h=32, w=32, d=64] fp32; full attention over
the 1024-position spatial grid independently per (b, head); output
[b, h, w, heads*d].

Sharding: data-parallel over batch — core c handles b=c (8 heads of
[1024, 64] attention per core), no cross-core communication.

Per-core pipeline (heads processed in PAIRS; matmuls bf16 with fp32 PSUM
accumulation). The PE executes serially on this toolchain, so the design
minimizes streamed columns + instruction count and keeps the HAM clock
gate warm (no transpose-heavy stretches > ~3.4us, dummy-matmul warm-up):
  - p-major seq tiling (seq = p*8 + t); ONE 4D casting DMA per (tensor,
    pair) interleaving the two heads -> 3 gpsimd triggers per pair
  - 40 dummy ident matmuls warm the PE clock gate (1.2 -> 2.4 GHz) while
    the first DMAs land
  - pair-interleaved natural tiles [128, t, 2, d]: one [128,128] PE
    transpose per block yields head A's d-rows on partitions 0:64 and
    B's on 64:128 — the packed pair layout mm1 wants
  - mm1 row-tiled: head A contracts on PE rows 0:64, head B on 64:128
    -> St [128, 1024] fp32 (separate tiles, freed by their own exp)
  - exp on ScalarE (activation Exp); optional per-(jb, head) offload to
    VectorE via the Schraudolph bit-trick (fused tensor_scalar
    mult+add -> int16 == bf16 exp approx) to unload the ScalarE
  - mm2: lhsT = [V | 1] j-chunk [128, 65] bf16, rhs = Pt slices ->
    accumulate PSUM Ot [65, 512] per i-half; row 64 = softmax sums
  - epilogue in bf16: ot copy on VectorE, PE transposes back (FWL), one
    batched reciprocal [128,4] + per-block tensor_scalar normalize into
    fp32 ostage, stores on the sync HWDGE ring
"""

from contextlib import ExitStack

import numpy as np

F32 = None
BF16 = None
I16 = None

_cache = {}

# Schraudolph exp in bf16 bit-space: bf16_bits(exp(s*x)) ~= round(x*A + B)
# A = s * 2^7/ln2, B = 2^7*(127 - sigma), sigma = 0.0430 balances the
# piecewise-linear 2^frac error (max rel err ~3%, mostly cancelling in the
# softmax average).
SCALE = 64.0 ** -0.5
SCHRAUD_A = SCALE * 128.0 / float(np.log(2.0))
SCHRAUD_B = 128.0 * (127.0 - 0.0430)

N_WARM = 40  # dummy matmuls to flip the PE HAM clock gate before real work

# (jb, head-in-pair) St tiles exp'd on VectorE via the Schraudolph bit-trick
# instead of ScalarE's exact exp. Each entry moves 1/16 of the attention
# weight mass to a ~3% weight-error approximation (mostly cancelling in the
# softmax average) and takes ~1.1us/pair off the ScalarE critical path.
SCHRAUD_TILES = frozenset()


def _imports():
    global F32, BF16, I16
    import concourse.bass as bass
    import concourse.tile as tile
    from concourse import mybir
    from concourse.masks import make_identity

    F32 = mybir.dt.float32
    BF16 = mybir.dt.bfloat16
    I16 = mybir.dt.int16
    return bass, tile, mybir, make_identity


def _split_multi_waits(nc, mybir):
    """Walrus in this container supports only ONE sync-wait per instruction.
    Hoist extra waits onto same-engine InstNoOp's inserted just before."""
    ctr = 0
    for f in nc.m.functions:
        for bb in f.blocks:
            insts = bb.instructions
            if not any(
                i.sync_info and i.sync_info.on_wait and len(i.sync_info.on_wait) > 1
                for i in insts
            ):
                continue
            out = []
            for inst in insts:
                si = inst.sync_info
                waits = list(si.on_wait) if si and si.on_wait else []
                if len(waits) > 1:
                    for w in waits[:-1]:
                        ctr += 1
                        nop = mybir.InstNoOp(
                            name=f"I-wsplit-{ctr}",
                            engine=inst.engine,
                            ins=[],
                            outs=[],
                            sync_info=mybir.SyncInfo(on_wait=[w], on_update=[]),
                        )
                        nc.register_instruction(nop)
                        out.append(nop)
                    si.on_wait = waits[-1:]
                out.append(inst)
            bb.instructions = out


def _build_nc(heads=8, seq=1024, d=64):
    bass, tile, mybir, make_identity = _imports()
    assert heads % 2 == 0 and seq == 1024 and d == 64
    nt = seq // 128          # 8 blocks of 128 positions
    nh = seq // 512          # 2 i-halves of 512
    dv = d + 1
    TS_MULT = mybir.AluOpType.mult
    TS_ADD = mybir.AluOpType.add

    nc = bass.Bass(trn_type="TRN2", target_bir_lowering=False)
    q_d = nc.dram_tensor("q", [heads, seq, d], F32, kind="ExternalInput")
    k_d = nc.dram_tensor("k", [heads, seq, d], F32, kind="ExternalInput")
    v_d = nc.dram_tensor("v", [heads, seq, d], F32, kind="ExternalInput")
    o_d = nc.dram_tensor("out", [seq, heads * d], F32, kind="ExternalOutput")

    # p-major: seq = p*nt + t; per-(p, t) HBM runs are 256B contiguous
    q_ap = q_d[:].rearrange("n (p t) d -> n p t d", p=128)
    k_ap = k_d[:].rearrange("n (p t) d -> n p t d", p=128)
    v_ap = v_d[:].rearrange("n (p t) d -> n p t d", p=128)
    o_ap = o_d[:].rearrange("(p t) c -> p t c", p=128)

    with tile.TileContext(nc) as tc, ExitStack() as ctx:
        consts = ctx.enter_context(tc.tile_pool(name="consts", bufs=1))
        nat = ctx.enter_context(tc.tile_pool(name="nat", bufs=2))
        dmaj = ctx.enter_context(tc.tile_pool(name="dmaj", bufs=2))
        ptp = ctx.enter_context(tc.tile_pool(name="ptp", bufs=36))
        otp = ctx.enter_context(tc.tile_pool(name="otp", bufs=3))
        outp = ctx.enter_context(tc.tile_pool(name="outp", bufs=3))
        small = ctx.enter_context(tc.tile_pool(name="small", bufs=4))

        # PSUM banks: st 2x2 (0-3) + oacc/ob/warm 2x1 (4-5) + tp 2x1 (6-7)
        st_ps = ctx.enter_context(tc.tile_pool(name="st_ps", bufs=2, space="PSUM"))
        oa_ps = ctx.enter_context(tc.tile_pool(name="oa_ps", bufs=2, space="PSUM"))
        tp_ps = ctx.enter_context(tc.tile_pool(name="tp_ps", bufs=2, space="PSUM"))

        ident_bf = consts.tile([128, 128], BF16)
        make_identity(nc, ident_bf[:])

        # Pre-warm the PE clock gate with dependency-free matmuls that run
        # while the first input DMAs are in flight (~4.3us of PE busy).
        warm = oa_ps.tile([128, 128], F32, tag="oacc")
        for _ in range(N_WARM):
            nc.tensor.matmul(warm[:], ident_bf[:], ident_bf[:], start=True, stop=True)

        def load_and_transpose(pair):
            """DMA pair inputs (bf16 cast, one 4D DMA per tensor) and build
            packed d-major tiles: head A on partitions 0:64, head B on
            64:128 (one [128,128] PE transpose per block)."""
            st8 = {"heads": (2 * pair, 2 * pair + 1), "v": None, "pts": [],
                   "oacc": {}, "ostage": {}}
            # pair-interleaved natural tiles: [..., 2, d] with head A at
            # index 0 and head B at 1, so one [128, 128] PE transpose of a
            # block yields A's d-rows on partitions 0:64 and B's on 64:128.
            qp = nat.tile([128, nt, 2, d], BF16, tag="qp")
            kp = nat.tile([128, nt, 2, d], BF16, tag="kp")
            hh = nt // 2
            for idx, n in enumerate(st8["heads"]):
                if pair == 0:
                    # halved loads so the first transposes start early
                    for lo, hi in ((0, hh), (hh, nt)):
                        nc.gpsimd.dma_start(
                            out=qp[:, lo:hi, idx, :], in_=q_ap[n, :, lo:hi]
                        )
                        nc.gpsimd.dma_start(
                            out=kp[:, lo:hi, idx, :], in_=k_ap[n, :, lo:hi]
                        )
                else:
                    nc.gpsimd.dma_start(out=qp[:, :, idx, :], in_=q_ap[n])
                    nc.gpsimd.dma_start(out=kp[:, :, idx, :], in_=k_ap[n])
            vp = nat.tile([128, nt, 2, dv], BF16, tag="vp")
            # ones columns for the softmax-denominator trick
            nc.vector.memset(vp[:, :, :, d : d + 1], 1.0)
            for idx, n in enumerate(st8["heads"]):
                nc.gpsimd.dma_start(out=vp[:, :, idx, 0:d], in_=v_ap[n])
            st8["v"] = vp
            qt = dmaj.tile([128, seq], BF16, tag="qt")
            kt = dmaj.tile([128, seq], BF16, tag="kt")
            for g in range(nt // 4):
                for src, dst in ((qp, qt), (kp, kt)):
                    tp = tp_ps.tile([128, 512], BF16, tag="tp")
                    for u in range(4):
                        t = g * 4 + u
                        nc.tensor.transpose(
                            tp[:, u * 128 : (u + 1) * 128],
                            src[:, t, :, :],
                            ident_bf[:],
                        )
                    nc.vector.tensor_copy(
                        out=dst[:, g * 512 : (g + 1) * 512], in_=tp[:]
                    )
            st8["qt"], st8["kt"] = qt, kt
            return st8

        def mm1_exp(s, jb):
            """Row-tiled pair mm1 into per-head St tiles + per-head exp.
            Separate St tiles mean head A's tile is released as soon as its
            own exp finishes."""
            qt, kt = s["qt"], s["kt"]
            stA = st_ps.tile([128, seq], F32, name="stA", tag="st")
            stB = st_ps.tile([128, seq], F32, name="stB", tag="st")
            for c in range(nh):
                nc.tensor.matmul(
                    stA[:, c * 512 : (c + 1) * 512],
                    kt[0:64, jb * 128 : (jb + 1) * 128],
                    qt[0:64, c * 512 : (c + 1) * 512],
                    start=True,
                    stop=True,
                )
                nc.tensor.matmul(
                    stB[:, c * 512 : (c + 1) * 512],
                    kt[64:128, jb * 128 : (jb + 1) * 128],
                    qt[64:128, c * 512 : (c + 1) * 512],
                    start=True,
                    stop=True,
                )
            for idx, st in enumerate((stA, stB)):
                pt = ptp.tile([128, seq], BF16, name="pt", tag="pt")
                if (jb, idx) in SCHRAUD_TILES:
                    nc.vector.tensor_scalar(
                        out=pt[:].bitcast(I16),
                        in0=st[:],
                        scalar1=SCHRAUD_A,
                        scalar2=SCHRAUD_B,
                        op0=TS_MULT,
                        op1=TS_ADD,
                    )
                else:
                    nc.scalar.activation(
                        out=pt[:],
                        in_=st[:],
                        func=mybir.ActivationFunctionType.Exp,
                        scale=SCALE,
                    )
                s["pts"].append((jb, idx, pt))

        def mm2_slot(s, slot):
            """One PE-stream slot of the pair's mm2: 4 accumulating
            matmuls of group (head, half) = slot//2; epilogue on the
            closing slot."""
            g = slot // 2
            phase = slot % 2
            idx, half = g // 2, g % 2
            if phase == 0:
                s["oacc"][g] = oa_ps.tile([dv, 512], F32, name="oacc", tag="oacc")
            oacc = s["oacc"][g]
            off = half * 512
            for jj in range(4):
                jb = phase * 4 + jj
                jb2, idx2, pt = s["pts"][jb * 2 + idx]
                assert jb2 == jb and idx2 == idx
                nc.tensor.matmul(
                    oacc[:],
                    s["v"][:, jb, idx, :],
                    pt[:, off : off + 512],
                    start=(jb == 0),
                    stop=(jb == nt - 1),
                )
            if phase == 1:
                _epilogue(s, idx, half, oacc)

        def _epilogue(s, idx, half, oacc):
            n = s["heads"][idx]
            if idx not in s["ostage"]:
                s["ostage"][idx] = outp.tile(
                    [128, nt, d], F32, name="ostage", tag="ostage"
                )
            ostage = s["ostage"][idx]
            ot = otp.tile([dv, 512], BF16, tag="ot")
            nc.vector.tensor_copy(out=ot[:], in_=oacc[:])
            # ob shares the oacc pool banks: rotation interleaves
            # oacc(g) -> ob(g) -> oacc(g+1), each WAR-safe by then.
            # dv+1 padding keeps each transpose's PSUM write 4B-aligned.
            ob = oa_ps.tile([128, 4, dv + 1], BF16, tag="oacc")
            for u in range(4):
                nc.tensor.transpose(
                    ob[:, u, 0:dv],
                    ot[:, u * 128 : (u + 1) * 128],
                    ident_bf[0:dv, 0:dv],
                )
            rec = small.tile([128, 4], F32, tag="rec")
            nc.vector.reciprocal(out=rec[:], in_=ob[:, :, d])
            for u in range(4):
                t = half * 4 + u
                nc.vector.tensor_scalar_mul(
                    ostage[:, t, :], ob[:, u, 0:d], rec[:, u : u + 1]
                )
            if half == nh - 1:
                nc.sync.dma_start(
                    out=o_ap[:, :, n * d : (n + 1) * d], in_=ostage[:]
                )

        # software pipeline: pair p's mm1/exp interleaved with pair p-1's mm2
        prev = None
        for pair in range(heads // 2):
            cur = load_and_transpose(pair)
            for jb in range(nt):
                mm1_exp(cur, jb)
                if prev is not None:
                    mm2_slot(prev, jb)
            prev = cur
        for jb in range(nt):
            mm2_slot(prev, jb)

    _split_multi_waits(nc, mybir)
    return nc


def _get_nc():
    if "nc" not in _cache:
        _cache["nc"] = _build_nc()
    return _cache["nc"]


def _run(q, k, v, trace=False):
    from concourse.bass_utils import run_bass_kernel_spmd

    b, heads, h, w, d = 8, 8, 32, 32, 64
    q = np.ascontiguousarray(np.asarray(q, dtype=np.float32))
    k = np.ascontiguousarray(np.asarray(k, dtype=np.float32))
    v = np.ascontiguousarray(np.asarray(v, dtype=np.float32))
    assert q.shape == (b, heads, h, w, d), q.shape

    nc = _get_nc()
    in_maps = [
        {
            "q": q[c].reshape(heads, h * w, d),
            "k": k[c].reshape(heads, h * w, d),
            "v": v[c].reshape(heads, h * w, d),
        }
        for c in range(b)
    ]
    res = run_bass_kernel_spmd(nc, in_maps, core_ids=list(range(b)), trace=trace)
    out = np.stack(
        [res.results[c]["out"].reshape(h, w, heads * d) for c in range(b)]
    )
    return out, res


def kernel(q, k, v):
    out, _ = _run(q, k, v)
    return out


# revision 23
# speedup vs baseline: 1.2655x; 1.1408x over previous
"""Trainium2 Bass kernel for multi-head quadratic spatial attention.

Problem: q,k,v [b=8, heads=8, h=32, w=32, d=64] fp32; full attention over
the 1024-position spatial grid independently per (b, head); output
[b, h, w, heads*d].

Sharding: data-parallel over batch — core c handles b=c (8 heads of
[1024, 64] attention per core), no cross-core communication.

Per-core pipeline (heads processed in PAIRS; matmuls bf16 with fp32 PSUM
accumulation). The PE executes serially on this toolchain, so the design
minimizes streamed columns + instruction count and keeps the HAM clock
gate warm (no transpose-heavy stretches > ~3.4us, dummy-matmul warm-up):
  - p-major seq tiling (seq = p*8 + t); ONE 4D casting DMA per (tensor,
    pair) interleaving the two heads -> 3 gpsimd triggers per pair
  - 40 dummy ident matmuls warm the PE clock gate (1.2 -> 2.4 GHz) while
    the first DMAs land
  - pair-interleaved natural tiles [128, t, 2, d]: one [128,128] PE
    transpose per block yields head A's d-rows on partitions 0:64 and
    B's on 64:128 — the packed pair layout mm1 wants
  - mm1 row-tiled: head A contracts on PE rows 0:64, head B on 64:128
    -> St [128, 1024] fp32 (separate tiles, freed by their own exp)
  - exp on ScalarE (activation Exp); optional per-(jb, head) offload to
    VectorE via the Schraudolph bit-trick (fused tensor_scalar
    mult+add -> int16 == bf16 exp approx) to unload the ScalarE
  - mm2: lhsT = [V | 1] j-chunk [128, 65] bf16, rhs = Pt slices ->
    accumulate PSUM Ot [65, 512] per i-half; row 64 = softmax sums
  - epilogue in bf16: ot copy on VectorE, PE transposes back (FWL), one
    batched reciprocal [128,4] + per-block tensor_scalar normalize into
    fp32 ostage, stores on the sync HWDGE ring
"""

from contextlib import ExitStack

import numpy as np

F32 = None
BF16 = None
I16 = None

_cache = {}

# Schraudolph exp in bf16 bit-space: bf16_bits(exp(s*x)) ~= round(x*A + B)
# A = s * 2^7/ln2, B = 2^7*(127 - sigma), sigma = 0.0430 balances the
# piecewise-linear 2^frac error (max rel err ~3%, mostly cancelling in the
# softmax average).
SCALE = 64.0 ** -0.5
SCHRAUD_A = SCALE * 128.0 / float(np.log(2.0))
SCHRAUD_B = 128.0 * (127.0 - 0.0430)

N_WARM = 40  # dummy matmuls to flip the PE HAM clock gate before real work

# (jb, head-in-pair) St tiles exp'd on VectorE via the Schraudolph bit-trick
# instead of ScalarE's exact exp. Each entry moves 1/16 of the attention
# weight mass to a ~3% weight-error approximation (mostly cancelling in the
# softmax average) and takes ~1.1us/pair off the ScalarE critical path.
SCHRAUD_TILES = frozenset()


def _imports():
    global F32, BF16, I16
    import concourse.bass as bass
    import concourse.tile as tile
    from concourse import mybir
    from concourse.masks import make_identity

    F32 = mybir.dt.float32
    BF16 = mybir.dt.bfloat16
    I16 = mybir.dt.int16
    return bass, tile, mybir, make_identity


def _split_multi_waits(nc, mybir):
    """Walrus in this container supports only ONE sync-wait per instruction.
    Hoist extra waits onto same-engine InstNoOp's inserted just before."""
    ctr = 0
    for f in nc.m.functions:
        for bb in f.blocks:
            insts = bb.instructions
            if not any(
                i.sync_info and i.sync_info.on_wait and len(i.sync_info.on_wait) > 1
                for i in insts
            ):
                continue
            out = []
            for inst in insts:
                si = inst.sync_info
                waits = list(si.on_wait) if si and si.on_wait else []
                if len(waits) > 1:
                    for w in waits[:-1]:
                        ctr += 1
                        nop = mybir.InstNoOp(
                            name=f"I-wsplit-{ctr}",
                            engine=inst.engine,
                            ins=[],
                            outs=[],
                            sync_info=mybir.SyncInfo(on_wait=[w], on_update=[]),
                        )
                        nc.register_instruction(nop)
                        out.append(nop)
                    si.on_wait = waits[-1:]
                out.append(inst)
            bb.instructions = out


def _build_nc(heads=8, seq=1024, d=64):
    bass, tile, mybir, make_identity = _imports()
    assert heads % 2 == 0 and seq == 1024 and d == 64
    nt = seq // 128          # 8 blocks of 128 positions
    nh = seq // 512          # 2 i-halves of 512
    dv = d + 1
    TS_MULT = mybir.AluOpType.mult
    TS_ADD = mybir.AluOpType.add

    nc = bass.Bass(trn_type="TRN2", target_bir_lowering=False)
    q_d = nc.dram_tensor("q", [heads, seq, d], F32, kind="ExternalInput")
    k_d = nc.dram_tensor("k", [heads, seq, d], F32, kind="ExternalInput")
    v_d = nc.dram_tensor("v", [heads, seq, d], F32, kind="ExternalInput")
    o_d = nc.dram_tensor("out", [seq, heads * d], F32, kind="ExternalOutput")

    # p-major: seq = p*nt + t; per-(p, t) HBM runs are 256B contiguous
    q_ap = q_d[:].rearrange("n (p t) d -> n p t d", p=128)
    k_ap = k_d[:].rearrange("n (p t) d -> n p t d", p=128)
    v_ap = v_d[:].rearrange("n (p t) d -> n p t d", p=128)
    o_ap = o_d[:].rearrange("(p t) c -> p t c", p=128)

    with tile.TileContext(nc) as tc, ExitStack() as ctx:
        consts = ctx.enter_context(tc.tile_pool(name="consts", bufs=1))
        nat = ctx.enter_context(tc.tile_pool(name="nat", bufs=2))
        dmaj = ctx.enter_context(tc.tile_pool(name="dmaj", bufs=2))
        ptp = ctx.enter_context(tc.tile_pool(name="ptp", bufs=36))
        otp = ctx.enter_context(tc.tile_pool(name="otp", bufs=3))
        outp = ctx.enter_context(tc.tile_pool(name="outp", bufs=3))
        small = ctx.enter_context(tc.tile_pool(name="small", bufs=4))

        # PSUM banks: st 2x2 (0-3) + oacc/ob/warm 2x1 (4-5) + tp 2x1 (6-7)
        st_ps = ctx.enter_context(tc.tile_pool(name="st_ps", bufs=2, space="PSUM"))
        oa_ps = ctx.enter_context(tc.tile_pool(name="oa_ps", bufs=2, space="PSUM"))
        tp_ps = ctx.enter_context(tc.tile_pool(name="tp_ps", bufs=2, space="PSUM"))

        ident_bf = consts.tile([128, 128], BF16)
        make_identity(nc, ident_bf[:])

        # Warm-up / filler matmuls keep the PE HAM clock gate at 2.4 GHz:
        # an idle (or transpose-only) stretch > ~3.4us re-throttles the PE
        # clock to 1.2 GHz for the next several microseconds. wsrc is
        # memset-ready within ~200ns of kernel start.
        wsrc = consts.tile([128, 128], BF16)
        nc.vector.memset(wsrc[:], 0.25)
        warm = oa_ps.tile([128, 128], F32, tag="oacc")

        def pe_filler(n):
            for _ in range(n):
                nc.tensor.matmul(
                    warm[:], wsrc[:], wsrc[:], start=True, stop=True
                )

        pe_filler(N_WARM)

        def load_and_transpose(pair):
            """DMA pair inputs (bf16 cast, one 4D DMA per tensor) and build
            packed d-major tiles: head A on partitions 0:64, head B on
            64:128 (one [128,128] PE transpose per block)."""
            st8 = {"heads": (2 * pair, 2 * pair + 1), "v": None, "pts": [],
                   "oacc": {}, "ostage": {}}
            # pair-interleaved natural tiles: [..., 2, d] with head A at
            # index 0 and head B at 1, so one [128, 128] PE transpose of a
            # block yields A's d-rows on partitions 0:64 and B's on 64:128.
            qp = nat.tile([128, nt, 2, d], BF16, tag="qp")
            kp = nat.tile([128, nt, 2, d], BF16, tag="kp")
            hh = nt // 2
            for idx, n in enumerate(st8["heads"]):
                if pair == 0:
                    # halved loads so the first transposes start early
                    for lo, hi in ((0, hh), (hh, nt)):
                        nc.gpsimd.dma_start(
                            out=qp[:, lo:hi, idx, :], in_=q_ap[n, :, lo:hi]
                        )
                        nc.gpsimd.dma_start(
                            out=kp[:, lo:hi, idx, :], in_=k_ap[n, :, lo:hi]
                        )
                else:
                    nc.gpsimd.dma_start(out=qp[:, :, idx, :], in_=q_ap[n])
                    nc.gpsimd.dma_start(out=kp[:, :, idx, :], in_=k_ap[n])
            vp = nat.tile([128, nt, 2, dv], BF16, tag="vp")
            # ones columns for the softmax-denominator trick
            nc.vector.memset(vp[:, :, :, d : d + 1], 1.0)
            for idx, n in enumerate(st8["heads"]):
                nc.gpsimd.dma_start(out=vp[:, :, idx, 0:d], in_=v_ap[n])
            st8["v"] = vp
            qt = dmaj.tile([128, seq], BF16, tag="qt")
            kt = dmaj.tile([128, seq], BF16, tag="kt")
            for g in range(nt // 4):
                for src, dst in ((qp, qt), (kp, kt)):
                    tp = tp_ps.tile([128, 512], BF16, tag="tp")
                    for u in range(4):
                        t = g * 4 + u
                        nc.tensor.transpose(
                            tp[:, u * 128 : (u + 1) * 128],
                            src[:, t, :, :],
                            ident_bf[:],
                        )
                    nc.vector.tensor_copy(
                        out=dst[:, g * 512 : (g + 1) * 512], in_=tp[:]
                    )
            st8["qt"], st8["kt"] = qt, kt
            return st8

        def mm1_exp(s, jb):
            """Row-tiled pair mm1 into per-head St tiles + per-head exp.
            Separate St tiles mean head A's tile is released as soon as its
            own exp finishes."""
            qt, kt = s["qt"], s["kt"]
            stA = st_ps.tile([128, seq], F32, name="stA", tag="st")
            stB = st_ps.tile([128, seq], F32, name="stB", tag="st")
            for c in range(nh):
                nc.tensor.matmul(
                    stA[:, c * 512 : (c + 1) * 512],
                    kt[0:64, jb * 128 : (jb + 1) * 128],
                    qt[0:64, c * 512 : (c + 1) * 512],
                    start=True,
                    stop=True,
                )
                nc.tensor.matmul(
                    stB[:, c * 512 : (c + 1) * 512],
                    kt[64:128, jb * 128 : (jb + 1) * 128],
                    qt[64:128, c * 512 : (c + 1) * 512],
                    start=True,
                    stop=True,
                )
            for idx, st in enumerate((stA, stB)):
                pt = ptp.tile([128, seq], BF16, name="pt", tag="pt")
                if (jb, idx) in SCHRAUD_TILES:
                    nc.vector.tensor_scalar(
                        out=pt[:].bitcast(I16),
                        in0=st[:],
                        scalar1=SCHRAUD_A,
                        scalar2=SCHRAUD_B,
                        op0=TS_MULT,
                        op1=TS_ADD,
                    )
                else:
                    nc.scalar.activation(
                        out=pt[:],
                        in_=st[:],
                        func=mybir.ActivationFunctionType.Exp,
                        scale=SCALE,
                    )
                s["pts"].append((jb, idx, pt))

        # slot s -> (group, phase); phase-1 slots sit at s>=2 so they only
        # run once all 8 jbs' Pt tiles exist (6-jb-shifted pipeline), while
        # at most 2 groups' oacc accumulators are ever live.
        SLOT_ORDER = [(0, 0), (1, 0), (0, 1), (1, 1), (2, 0), (3, 0), (2, 1), (3, 1)]

        def mm2_slot(s, slot):
            """One PE-stream slot of the pair's mm2: 4 accumulating
            matmuls of a (head, half) group; epilogue on the closing
            phase."""
            g, phase = SLOT_ORDER[slot]
            idx, half = g // 2, g % 2
            if phase == 0:
                s["oacc"][g] = oa_ps.tile([dv, 512], F32, name="oacc", tag="oacc")
            oacc = s["oacc"][g]
            off = half * 512
            for jj in range(4):
                jb = phase * 4 + jj
                jb2, idx2, pt = s["pts"][jb * 2 + idx]
                assert jb2 == jb and idx2 == idx
                nc.tensor.matmul(
                    oacc[:],
                    s["v"][:, jb, idx, :],
                    pt[:, off : off + 512],
                    start=(jb == 0),
                    stop=(jb == nt - 1),
                )
            if phase == 1:
                _epilogue(s, idx, half, oacc)

        def _epilogue(s, idx, half, oacc):
            n = s["heads"][idx]
            if idx not in s["ostage"]:
                s["ostage"][idx] = outp.tile(
                    [128, nt, d], F32, name="ostage", tag="ostage"
                )
            ostage = s["ostage"][idx]
            ot = otp.tile([dv, 512], BF16, tag="ot")
            nc.vector.tensor_copy(out=ot[:], in_=oacc[:])
            # ob shares the oacc pool banks: rotation interleaves
            # oacc(g) -> ob(g) -> oacc(g+1), each WAR-safe by then.
            # dv+1 padding keeps each transpose's PSUM write 4B-aligned.
            ob = oa_ps.tile([128, 4, dv + 1], BF16, tag="oacc")
            for u in range(4):
                nc.tensor.transpose(
                    ob[:, u, 0:dv],
                    ot[:, u * 128 : (u + 1) * 128],
                    ident_bf[0:dv, 0:dv],
                )
            rec = small.tile([128, 4], F32, tag="rec")
            nc.vector.reciprocal(out=rec[:], in_=ob[:, :, d])
            for u in range(4):
                t = half * 4 + u
                nc.vector.tensor_scalar_mul(
                    ostage[:, t, :], ob[:, u, 0:d], rec[:, u : u + 1]
                )
            if half == nh - 1:
                nc.sync.dma_start(
                    out=o_ap[:, :, n * d : (n + 1) * d], in_=ostage[:]
                )

        # software pipeline, 6-jb shifted: mm2 slot S runs alongside mm1 of
        # global jb S+6, so pair 0's loop is mm2-dense from jb 6 (instead
        # of mm1-only for a whole pair) and the mm2-only tail is 6 slots.
        SHIFT = 6
        states = []
        for pair in range(heads // 2):
            cur = load_and_transpose(pair)
            states.append(cur)
            for jb in range(nt):
                mm1_exp(cur, jb)
                S = pair * nt + jb - SHIFT
                if S >= 0:
                    mm2_slot(states[S // nt], S % nt)
                else:
                    # keep the PE clock gate warm through the fill phase
                    pe_filler(6)
        for S in range(heads // 2 * nt - SHIFT, heads // 2 * nt):
            mm2_slot(states[S // nt], S % nt)

    _split_multi_waits(nc, mybir)
    return nc


def _get_nc():
    if "nc" not in _cache:
        _cache["nc"] = _build_nc()
    return _cache["nc"]


def _run(q, k, v, trace=False):
    from concourse.bass_utils import run_bass_kernel_spmd

    b, heads, h, w, d = 8, 8, 32, 32, 64
    q = np.ascontiguousarray(np.asarray(q, dtype=np.float32))
    k = np.ascontiguousarray(np.asarray(k, dtype=np.float32))
    v = np.ascontiguousarray(np.asarray(v, dtype=np.float32))
    assert q.shape == (b, heads, h, w, d), q.shape

    nc = _get_nc()
    in_maps = [
        {
            "q": q[c].reshape(heads, h * w, d),
            "k": k[c].reshape(heads, h * w, d),
            "v": v[c].reshape(heads, h * w, d),
        }
        for c in range(b)
    ]
    res = run_bass_kernel_spmd(nc, in_maps, core_ids=list(range(b)), trace=trace)
    out = np.stack(
        [res.results[c]["out"].reshape(h, w, heads * d) for c in range(b)]
    )
    return out, res


def kernel(q, k, v):
    out, _ = _run(q, k, v)
    return out
